# revision 5
# baseline (speedup 1.0000x reference)
"""Multi-head attention (B=2, S=2048, RES=1024, H=16) on 8 NeuronCores.

Sharding: batch*heads across cores. Core c handles batch c//4 and heads
4*(c%4) .. 4*(c%4)+3 (column-sharded QKV weights). No cross-core comm.

Per-core kernel (S=2048, K=1024, C=256 = 4 heads x 64):
  xT = transpose(x_b)                       via PE transpose, f32r
  QT = (Wq_c)^T x_b^T   [C, S]             f32r matmuls, K on partitions
  KT = (Wk_c)^T x_b^T   [C, S]
  V  = x_b Wv_c         [S, C] (+ ones col per head for softmax sums)
  per head h: scoresT[t,s] = K_h^T Q_h / 8 -> exp on ACT -> attnT
              outT[d,s] (+ sums row via ones col) = V_aug^T attnT
              transpose outT back, multiply by 1/sums, DMA out.
"""

import sys

if "/opt/trn_rl_repo" not in sys.path:
    sys.path.insert(0, "/opt/trn_rl_repo")

import numpy as np

B = 2
S = 2048
RES = 1024
HEADS = 16
HD = 64  # head dim
N_CORES = 8
HPC = 4  # heads per core
C = HPC * HD  # 256 per-core projected width
K = RES  # contraction dim of projections
NKT = K // 128  # 8 k-chunks
NST = S // 128  # 16 s-tiles / t-blocks
SH = 1024  # s-half size for attention inner loop
VAUG = HD + 2  # 66: V cols + ones col + zero pad (fp32r needs even dims)

_CACHE: dict = {}


def _build_nc():
    import concourse.mybir as mybir
    import concourse.tile as tile
    from concourse import bacc
    from concourse.masks import make_identity

    f32 = mybir.dt.float32
    f32r = mybir.dt.float32r
    AF = mybir.ActivationFunctionType

    nc = bacc.Bacc(None)
    x_in = nc.dram_tensor("x", [S, K], f32r, kind="ExternalInput")
    wq_in = nc.dram_tensor("wq", [K, C], f32r, kind="ExternalInput")
    wk_in = nc.dram_tensor("wk", [K, C], f32r, kind="ExternalInput")
    wv_in = nc.dram_tensor("wv", [K, C], f32r, kind="ExternalInput")
    out_d = nc.dram_tensor("out", [S, C], f32, kind="ExternalOutput")

    with tile.TileContext(nc) as tc:
        with tc.tile_pool(name="persist", bufs=1) as persist:
            # identity for PE transposes
            ident32 = persist.tile([128, 128], f32)
            make_identity(nc, ident32)
            ident = persist.tile([128, 128], f32r)
            nc.vector.tensor_copy(ident[:], ident32[:])
            ones4 = persist.tile([128, HPC], f32)
            nc.vector.memset(ones4[:], 1.0)
            zeros4 = persist.tile([128, HPC], f32)
            nc.vector.memset(zeros4[:], 0.0)

            # Q^T / K^T tiles [128, S] per 128-wide block of C
            qt_tiles = []
            kt_tiles = []
            for cb in range(C // 128):
                qt = persist.tile([128, S], f32r, name=f"qt_{cb}", tag="qt", bufs=2)
                kt = persist.tile([128, S], f32r, name=f"kt_{cb}", tag="kt", bufs=2)
                qt_tiles.append(qt)
                kt_tiles.append(kt)

            # V tiles with ones column per head; head h at cols
            # h*VAUG .. h*VAUG+HD, ones at col h*VAUG+HD
            v_aug = []
            for st in range(NST):
                va = persist.tile(
                    [128, HPC * VAUG], f32r, name=f"vaug_{st}", tag="vaug", bufs=NST
                )
                v_aug.append(va)

            # output staging tiles [128, C], one per s-block
            out_tiles = []
            for sb in range(NST):
                ot = persist.tile([128, C], f32, name=f"out_{sb}", tag="ot", bufs=NST)
                out_tiles.append(ot)

            # ================= phase A/B: load, transpose, project ======
            with (
                tc.tile_pool(name="xw", bufs=1) as xw,
                tc.tile_pool(name="ps_pre", bufs=1, space="PSUM") as psp,
            ):
                wq_t = []
                wk_t = []
                wv_t = []
                for kk in range(NKT):
                    wq_kk = xw.tile([128, C], f32r, name=f"wq_{kk}", tag="wq", bufs=NKT)
                    nc.sync.dma_start(wq_kk[:], wq_in[kk * 128 : (kk + 1) * 128, :])
                    wq_t.append(wq_kk)
                    wk_kk = xw.tile([128, C], f32r, name=f"wk_{kk}", tag="wk", bufs=NKT)
                    nc.sync.dma_start(wk_kk[:], wk_in[kk * 128 : (kk + 1) * 128, :])
                    wk_t.append(wk_kk)
                    wv_kk = xw.tile([128, C], f32r, name=f"wv_{kk}", tag="wv", bufs=NKT)
                    nc.sync.dma_start(wv_kk[:], wv_in[kk * 128 : (kk + 1) * 128, :])
                    wv_t.append(wv_kk)

                # load x and transpose: xT [128(k), NKT, S]
                xT = xw.tile([128, NKT * S], f32r, name="xT")
                xT3 = xT.rearrange("p (k s) -> p k s", k=NKT)
                for st in range(NST):
                    x_t = xw.tile([128, K], f32r, name=f"x_{st}", tag="xload", bufs=2)
                    nc.sync.dma_start(x_t[:], x_in[st * 128 : (st + 1) * 128, :])
                    for kg in range(NKT // 4):
                        tr_ps = psp.tile(
                            [128, 512], f32r, name=f"xtr_{st}_{kg}", tag="xtr", bufs=2
                        )
                        for j in range(4):
                            kk = kg * 4 + j
                            nc.tensor.transpose(
                                tr_ps[:, j * 128 : (j + 1) * 128],
                                x_t[:, kk * 128 : (kk + 1) * 128],
                                ident[:],
                            )
                        nc.vector.tensor_copy(
                            xT3[:, kg * 4 : (kg + 1) * 4, st * 128 : (st + 1) * 128],
                            tr_ps.rearrange("p (j b) -> p j b", j=4),
                        )

                # Q^T and K^T projections
                for cb in range(C // 128):
                    for w_t, dst in ((wq_t, qt_tiles[cb]), (wk_t, kt_tiles[cb])):
                        for sc in range(S // 512):
                            pp = psp.tile(
                                [128, 512],
                                f32,
                                name=f"proj_{cb}_{sc}",
                                tag="proj",
                                bufs=2,
                            )
                            for kk in range(NKT):
                                nc.tensor.matmul(
                                    pp[:],
                                    w_t[kk][:, cb * 128 : (cb + 1) * 128],
                                    xT3[:, kk, sc * 512 : (sc + 1) * 512],
                                    start=(kk == 0),
                                    stop=(kk == NKT - 1),
                                )
                            nc.vector.tensor_copy(
                                dst[:, sc * 512 : (sc + 1) * 512], pp[:]
                            )

                # V projection into augmented tiles
                for st in range(NST):
                    va3 = v_aug[st].rearrange("p (h d) -> p h d", h=HPC)
                    vp = psp.tile([128, C], f32, name=f"vproj_{st}", tag="vproj", bufs=2)
                    for kk in range(NKT):
                        nc.tensor.matmul(
                            vp[:],
                            xT3[:, kk, st * 128 : (st + 1) * 128],
                            wv_t[kk][:],
                            start=(kk == 0),
                            stop=(kk == NKT - 1),
                        )
                    nc.vector.tensor_copy(
                        va3[:, :, 0:HD], vp.rearrange("p (h d) -> p h d", h=HPC)
                    )
                    nc.vector.tensor_copy(
                        va3[:, :, HD : HD + 1],
                        ones4.rearrange("p (h o) -> p h o", h=HPC),
                    )
                    nc.vector.tensor_copy(
                        va3[:, :, HD + 1 : HD + 2],
                        zeros4.rearrange("p (h o) -> p h o", h=HPC),
                    )

            # ================= attention =================
            with (
                tc.tile_pool(name="attn", bufs=2) as attn,
                tc.tile_pool(name="ps_attn", bufs=2, space="PSUM") as psa,
            ):
                for hp in range(HPC // 2):
                    qt = qt_tiles[hp]
                    kt = kt_tiles[hp]
                    for shi in range(S // SH):
                        s0 = shi * SH
                        outp = []  # psum [VAUG, SH] per head in pair
                        for side in range(2):
                            op = psa.tile(
                                [VAUG, SH],
                                f32,
                                name=f"outT_{hp}_{shi}_{side}",
                                tag="outT",
                                bufs=2,
                            )
                            outp.append(op)
                        for t in range(NST):
                            att = []
                            for side in range(2):
                                dlo = side * HD
                                dhi = dlo + HD
                                sc_ps = psa.tile(
                                    [128, SH],
                                    f32,
                                    name=f"sc_{hp}_{shi}_{t}_{side}",
                                    tag="sc",
                                    bufs=2,
                                )
                                for scj in range(SH // 512):
                                    nc.tensor.matmul(
                                        sc_ps[:, scj * 512 : (scj + 1) * 512],
                                        kt[dlo:dhi, t * 128 : (t + 1) * 128],
                                        qt[
                                            dlo:dhi,
                                            s0 + scj * 512 : s0 + (scj + 1) * 512,
                                        ],
                                        start=True,
                                        stop=True,
                                    )
                                at = attn.tile(
                                    [128, SH],
                                    f32r,
                                    name=f"at_{hp}_{shi}_{t}_{side}",
                                    tag=f"at{side}",
                                    bufs=2,
                                )
                                nc.scalar.activation(at[:], sc_ps[:], AF.Exp, scale=0.125)
                                att.append(at)
                            for side in range(2):
                                h_loc = 2 * hp + side
                                for scj in range(SH // 512):
                                    nc.tensor.matmul(
                                        outp[side][:, scj * 512 : (scj + 1) * 512],
                                        v_aug[t][:, h_loc * VAUG : (h_loc + 1) * VAUG],
                                        att[side][:, scj * 512 : (scj + 1) * 512],
                                        start=(t == 0),
                                        stop=(t == NST - 1),
                                    )
                        # tail: copy both outT psums to SBUF first (frees psum),
                        # then transpose back, normalize, stage output
                        oTs = []
                        for side in range(2):
                            oT = attn.tile(
                                [VAUG, SH],
                                f32r,
                                name=f"oT_{hp}_{shi}_{side}",
                                tag="oT",
                                bufs=2,
                            )
                            nc.vector.tensor_copy(oT[:], outp[side][:])
                            oTs.append(oT)
                        for side in range(2):
                            h_loc = 2 * hp + side
                            for j in range(SH // 128):
                                sb = (s0 + j * 128) // 128
                                trp = psa.tile(
                                    [128, VAUG],
                                    f32r,
                                    name=f"trp_{hp}_{shi}_{side}_{j}",
                                    tag="outT",
                                    bufs=2,
                                )
                                nc.tensor.transpose(
                                    trp[:],
                                    oTs[side][:, j * 128 : (j + 1) * 128],
                                    ident[0:VAUG, 0:VAUG],
                                )
                                rs = attn.tile(
                                    [128, 1],
                                    f32,
                                    name=f"rs_{hp}_{shi}_{side}_{j}",
                                    tag="rs",
                                    bufs=4,
                                )
                                nc.vector.reciprocal(rs[:], trp[:, HD : HD + 1])
                                nc.vector.tensor_scalar_mul(
                                    out_tiles[sb][:, h_loc * HD : (h_loc + 1) * HD],
                                    trp[:, 0:HD],
                                    rs[:],
                                )

                # write out
                for sb in range(NST):
                    nc.sync.dma_start(
                        out_d[sb * 128 : (sb + 1) * 128, :], out_tiles[sb][:]
                    )

    nc.finalize()
    return nc


def _get_nc():
    if "nc" not in _CACHE:
        _CACHE["nc"] = _build_nc()
    return _CACHE["nc"]


def kernel(x, Wq, Wk, Wv):
    from concourse import bass_utils

    x = np.asarray(x, dtype=np.float32)
    Wq = np.asarray(Wq, dtype=np.float32)
    Wk = np.asarray(Wk, dtype=np.float32)
    Wv = np.asarray(Wv, dtype=np.float32)

    nc = _get_nc()
    in_maps = []
    for c in range(N_CORES):
        b = c // 4
        g = c % 4
        cols = slice(g * C, (g + 1) * C)
        in_maps.append(
            {
                "x": np.ascontiguousarray(x[b]),
                "wq": np.ascontiguousarray(Wq[:, cols]),
                "wk": np.ascontiguousarray(Wk[:, cols]),
                "wv": np.ascontiguousarray(Wv[:, cols]),
            }
        )

    res = bass_utils.run_bass_kernel_spmd(nc, in_maps, list(range(N_CORES)))
    _CACHE["last_results"] = res

    out = np.empty((B, S, RES), dtype=np.float32)
    for c in range(N_CORES):
        b = c // 4
        g = c % 4
        out[b, :, g * C : (g + 1) * C] = res.results[c]["out"]
    return out


# revision 6
# speedup vs baseline: 1.2447x; 1.2447x over previous
"""Multi-head attention (B=2, S=2048, RES=1024, H=16) on 8 NeuronCores.

Sharding: batch*heads across cores. Core c handles batch c//4 and heads
4*(c%4) .. 4*(c%4)+3 (column-sharded QKV weights). No cross-core comm.

Per-core kernel (S=2048, K=1024, C=256 = 4 heads x 64):
  xT = transpose(x_b)                       via PE transpose, f32r
  QT = (Wq_c)^T x_b^T   [C, S]             f32r matmuls, K on partitions
  KT = (Wk_c)^T x_b^T   [C, S]
  V  = x_b Wv_c         [S, C] (+ ones col per head for softmax sums)
  per head h: scoresT[t,s] = K_h^T Q_h / 8 -> exp on ACT -> attnT
              outT[d,s] (+ sums row via ones col) = V_aug^T attnT
              transpose outT back, multiply by 1/sums, DMA out.
"""

import sys

if "/opt/trn_rl_repo" not in sys.path:
    sys.path.insert(0, "/opt/trn_rl_repo")

import numpy as np

B = 2
S = 2048
RES = 1024
HEADS = 16
HD = 64  # head dim
N_CORES = 8
HPC = 4  # heads per core
C = HPC * HD  # 256 per-core projected width
K = RES  # contraction dim of projections
NKT = K // 128  # 8 k-chunks
NST = S // 128  # 16 s-tiles / t-blocks
SH = 1024  # s-half size for attention inner loop
VAUG = HD + 2  # 66: V cols + ones col + zero pad (fp32r needs even dims)

_CACHE: dict = {}


def _build_nc():
    import concourse.mybir as mybir
    import concourse.tile as tile
    from concourse import bacc
    from concourse.masks import make_identity

    f32 = mybir.dt.float32
    f32r = mybir.dt.float32r
    bf16 = mybir.dt.bfloat16
    AF = mybir.ActivationFunctionType

    nc = bacc.Bacc(None)
    x_in = nc.dram_tensor("x", [S, K], bf16, kind="ExternalInput")
    wq_in = nc.dram_tensor("wq", [K, C], bf16, kind="ExternalInput")
    wk_in = nc.dram_tensor("wk", [K, C], bf16, kind="ExternalInput")
    wv_in = nc.dram_tensor("wv", [K, C], bf16, kind="ExternalInput")
    out_d = nc.dram_tensor("out", [S, C], f32, kind="ExternalOutput")

    with tile.TileContext(nc) as tc:
        with tc.tile_pool(name="persist", bufs=1) as persist:
            # identity for PE transposes
            ident32 = persist.tile([128, 128], f32)
            make_identity(nc, ident32)
            ident = persist.tile([128, 128], bf16)
            nc.vector.tensor_copy(ident[:], ident32[:])
            identr = persist.tile([128, 128], f32r)
            nc.vector.tensor_copy(identr[:], ident32[:])
            ones4 = persist.tile([128, HPC], f32)
            nc.vector.memset(ones4[:], 1.0)
            zeros4 = persist.tile([128, HPC], f32)
            nc.vector.memset(zeros4[:], 0.0)

            # Q^T / K^T tiles [128, S] per 128-wide block of C
            qt_tiles = []
            kt_tiles = []
            for cb in range(C // 128):
                qt = persist.tile([128, S], bf16, name=f"qt_{cb}", tag="qt", bufs=2)
                kt = persist.tile([128, S], bf16, name=f"kt_{cb}", tag="kt", bufs=2)
                qt_tiles.append(qt)
                kt_tiles.append(kt)

            # V tiles with ones column per head; head h at cols
            # h*VAUG .. h*VAUG+HD, ones at col h*VAUG+HD
            v_aug = []
            for st in range(NST):
                va = persist.tile(
                    [128, HPC * VAUG], bf16, name=f"vaug_{st}", tag="vaug", bufs=NST
                )
                v_aug.append(va)

            # output staging tiles [128, C], one per s-block
            out_tiles = []
            for sb in range(NST):
                ot = persist.tile([128, C], f32, name=f"out_{sb}", tag="ot", bufs=NST)
                out_tiles.append(ot)

            # ================= phase A/B: load, transpose, project ======
            with (
                tc.tile_pool(name="xw", bufs=1) as xw,
                tc.tile_pool(name="ps_pre", bufs=1, space="PSUM") as psp,
            ):
                wq_t = []
                wk_t = []
                wv_t = []
                for kk in range(NKT):
                    wq_kk = xw.tile([128, C], bf16, name=f"wq_{kk}", tag="wq", bufs=NKT)
                    nc.sync.dma_start(wq_kk[:], wq_in[kk * 128 : (kk + 1) * 128, :])
                    wq_t.append(wq_kk)
                    wk_kk = xw.tile([128, C], bf16, name=f"wk_{kk}", tag="wk", bufs=NKT)
                    nc.sync.dma_start(wk_kk[:], wk_in[kk * 128 : (kk + 1) * 128, :])
                    wk_t.append(wk_kk)
                    wv_kk = xw.tile([128, C], bf16, name=f"wv_{kk}", tag="wv", bufs=NKT)
                    nc.sync.dma_start(wv_kk[:], wv_in[kk * 128 : (kk + 1) * 128, :])
                    wv_t.append(wv_kk)

                # load x and transpose: xT [128(k), NKT, S]
                xT = xw.tile([128, NKT * S], bf16, name="xT")
                xT3 = xT.rearrange("p (k s) -> p k s", k=NKT)
                for st in range(NST):
                    x_t = xw.tile([128, K], bf16, name=f"x_{st}", tag="xload", bufs=2)
                    nc.sync.dma_start(x_t[:], x_in[st * 128 : (st + 1) * 128, :])
                    for kg in range(NKT // 4):
                        tr_ps = psp.tile(
                            [128, 512], bf16, name=f"xtr_{st}_{kg}", tag="xtr", bufs=2
                        )
                        for j in range(4):
                            kk = kg * 4 + j
                            nc.tensor.transpose(
                                tr_ps[:, j * 128 : (j + 1) * 128],
                                x_t[:, kk * 128 : (kk + 1) * 128],
                                ident[:],
                            )
                        nc.vector.tensor_copy(
                            xT3[:, kg * 4 : (kg + 1) * 4, st * 128 : (st + 1) * 128],
                            tr_ps.rearrange("p (j b) -> p j b", j=4),
                        )

                # Q^T and K^T projections
                for cb in range(C // 128):
                    for w_t, dst in ((wq_t, qt_tiles[cb]), (wk_t, kt_tiles[cb])):
                        for sc in range(S // 512):
                            pp = psp.tile(
                                [128, 512],
                                f32,
                                name=f"proj_{cb}_{sc}",
                                tag="proj",
                                bufs=2,
                            )
                            for kk in range(NKT):
                                nc.tensor.matmul(
                                    pp[:],
                                    w_t[kk][:, cb * 128 : (cb + 1) * 128],
                                    xT3[:, kk, sc * 512 : (sc + 1) * 512],
                                    start=(kk == 0),
                                    stop=(kk == NKT - 1),
                                )
                            nc.vector.tensor_copy(
                                dst[:, sc * 512 : (sc + 1) * 512], pp[:]
                            )

                # V projection into augmented tiles
                for st in range(NST):
                    va3 = v_aug[st].rearrange("p (h d) -> p h d", h=HPC)
                    vp = psp.tile([128, C], f32, name=f"vproj_{st}", tag="vproj", bufs=2)
                    for kk in range(NKT):
                        nc.tensor.matmul(
                            vp[:],
                            xT3[:, kk, st * 128 : (st + 1) * 128],
                            wv_t[kk][:],
                            start=(kk == 0),
                            stop=(kk == NKT - 1),
                        )
                    nc.vector.tensor_copy(
                        va3[:, :, 0:HD], vp.rearrange("p (h d) -> p h d", h=HPC)
                    )
                    nc.vector.tensor_copy(
                        va3[:, :, HD : HD + 1],
                        ones4.rearrange("p (h o) -> p h o", h=HPC),
                    )
                    nc.vector.tensor_copy(
                        va3[:, :, HD + 1 : HD + 2],
                        zeros4.rearrange("p (h o) -> p h o", h=HPC),
                    )

            # ================= attention =================
            with (
                tc.tile_pool(name="attn", bufs=2) as attn,
                tc.tile_pool(name="ps_attn", bufs=2, space="PSUM") as psa,
            ):
                for hp in range(HPC // 2):
                    qt = qt_tiles[hp]
                    kt = kt_tiles[hp]
                    for shi in range(S // SH):
                        s0 = shi * SH
                        outp = []  # psum [VAUG, SH] per head in pair
                        for side in range(2):
                            op = psa.tile(
                                [VAUG, SH],
                                f32,
                                name=f"outT_{hp}_{shi}_{side}",
                                tag="outT",
                                bufs=2,
                            )
                            outp.append(op)
                        for t in range(NST):
                            att = []
                            for side in range(2):
                                dlo = side * HD
                                dhi = dlo + HD
                                sc_ps = psa.tile(
                                    [128, SH],
                                    f32,
                                    name=f"sc_{hp}_{shi}_{t}_{side}",
                                    tag="sc",
                                    bufs=2,
                                )
                                for scj in range(SH // 512):
                                    nc.tensor.matmul(
                                        sc_ps[:, scj * 512 : (scj + 1) * 512],
                                        kt[dlo:dhi, t * 128 : (t + 1) * 128],
                                        qt[
                                            dlo:dhi,
                                            s0 + scj * 512 : s0 + (scj + 1) * 512,
                                        ],
                                        start=True,
                                        stop=True,
                                    )
                                at = attn.tile(
                                    [128, SH],
                                    bf16,
                                    name=f"at_{hp}_{shi}_{t}_{side}",
                                    tag=f"at{side}",
                                    bufs=2,
                                )
                                nc.scalar.activation(at[:], sc_ps[:], AF.Exp, scale=0.125)
                                att.append(at)
                            for side in range(2):
                                h_loc = 2 * hp + side
                                for scj in range(SH // 512):
                                    nc.tensor.matmul(
                                        outp[side][:, scj * 512 : (scj + 1) * 512],
                                        v_aug[t][:, h_loc * VAUG : (h_loc + 1) * VAUG],
                                        att[side][:, scj * 512 : (scj + 1) * 512],
                                        start=(t == 0),
                                        stop=(t == NST - 1),
                                    )
                        # tail: copy both outT psums to SBUF first (frees psum),
                        # then transpose back, normalize, stage output
                        oTs = []
                        for side in range(2):
                            oT = attn.tile(
                                [VAUG, SH],
                                f32r,
                                name=f"oT_{hp}_{shi}_{side}",
                                tag="oT",
                                bufs=2,
                            )
                            nc.vector.tensor_copy(oT[:], outp[side][:])
                            oTs.append(oT)
                        for side in range(2):
                            h_loc = 2 * hp + side
                            for j in range(SH // 128):
                                sb = (s0 + j * 128) // 128
                                trp = psa.tile(
                                    [128, VAUG],
                                    f32r,
                                    name=f"trp_{hp}_{shi}_{side}_{j}",
                                    tag="outT",
                                    bufs=2,
                                )
                                nc.tensor.transpose(
                                    trp[:],
                                    oTs[side][:, j * 128 : (j + 1) * 128],
                                    identr[0:VAUG, 0:VAUG],
                                )
                                rs = attn.tile(
                                    [128, 1],
                                    f32,
                                    name=f"rs_{hp}_{shi}_{side}_{j}",
                                    tag="rs",
                                    bufs=4,
                                )
                                nc.vector.reciprocal(rs[:], trp[:, HD : HD + 1])
                                nc.vector.tensor_scalar_mul(
                                    out_tiles[sb][:, h_loc * HD : (h_loc + 1) * HD],
                                    trp[:, 0:HD],
                                    rs[:],
                                )

                # write out
                for sb in range(NST):
                    nc.sync.dma_start(
                        out_d[sb * 128 : (sb + 1) * 128, :], out_tiles[sb][:]
                    )

    nc.finalize()
    return nc


def _get_nc():
    if "nc" not in _CACHE:
        _CACHE["nc"] = _build_nc()
    return _CACHE["nc"]


def kernel(x, Wq, Wk, Wv):
    import ml_dtypes
    from concourse import bass_utils

    bf = ml_dtypes.bfloat16
    x = np.asarray(x, dtype=np.float32).astype(bf)
    Wq = np.asarray(Wq, dtype=np.float32).astype(bf)
    Wk = np.asarray(Wk, dtype=np.float32).astype(bf)
    Wv = np.asarray(Wv, dtype=np.float32).astype(bf)

    nc = _get_nc()
    in_maps = []
    for c in range(N_CORES):
        b = c // 4
        g = c % 4
        cols = slice(g * C, (g + 1) * C)
        in_maps.append(
            {
                "x": np.ascontiguousarray(x[b]),
                "wq": np.ascontiguousarray(Wq[:, cols]),
                "wk": np.ascontiguousarray(Wk[:, cols]),
                "wv": np.ascontiguousarray(Wv[:, cols]),
            }
        )

    res = bass_utils.run_bass_kernel_spmd(nc, in_maps, list(range(N_CORES)))
    _CACHE["last_results"] = res

    out = np.empty((B, S, RES), dtype=np.float32)
    for c in range(N_CORES):
        b = c // 4
        g = c % 4
        out[b, :, g * C : (g + 1) * C] = res.results[c]["out"]
    return out
